# revision 8
# baseline (speedup 1.0000x reference)
"""ChromDropout kernel for one TRN2 chip (8 NeuronCores, data-parallel).

Math (training-mode ChromDropout):
    out[b, g] = x[b, g] * (1 - drop[b, chrom_ids[g]]) * (NUM_CHROMS / N_DROP)
where drop[b, :] marks 4 distinct chromosomes sampled per row with
jax.random.permutation(split(key(42), B)[b], 23)[:4].

Strategy:
  - Host (tiny): derive the per-row drop table exactly as the reference does
    (threefry is platform-deterministic), fold the 5.75 rescale into a
    [B, 23] scale table with values {0, 5.75}, and build a [23, G] one-hot
    of chrom_ids. Both are exact in bf16 (0, 1, 5.75), so they ship as bf16
    for full-rate TensorEngine matmuls and the output stays bit-identical.
  - Device (all the heavy traffic): shard x row-wise across 8 cores
    (2048 rows each). 2048 rows = 128 partitions x 16 rows, so each SBUF
    tile holds one full row per partition (38-76KB contiguous DMA runs).
    Per tile, mask[128, N] = scale_perm[23, 128].T @ onehot[23, N] on the
    TensorEngine (exact one-hot selection), then out = x * mask on the
    VectorEngine. Streaming, memory-bound by design.
"""

import numpy as np

B = 16384
G = 19064
C = 23
ND = 4
SCALE = float(C) / float(ND)  # 5.75, exactly representable in bf16
NCORES = 8
BS = B // NCORES  # 2048 rows per core
P = 128
QW = BS // P  # 16 rows per partition
W = G // 2  # 9532: half a row per x tile
MM = 512  # matmul moving-free-dim chunk (one PSUM bank of f32)

_CACHED = {}


def _build_nc():
    import concourse.bacc as bacc
    import concourse.mybir as mybir
    from concourse.tile import TileContext

    f32 = mybir.dt.float32
    bf16 = mybir.dt.bfloat16
    # Bacc (not raw Bass): its compile() runs move_matmul_waits_to_ldweights +
    # generate_event_semaphores, which split multi-semaphore waits down to the
    # 1-wait-per-instruction TRN2 ISA limit.
    nc = bacc.Bacc("TRN2", target_bir_lowering=False, debug=False)
    x = nc.declare_dram_parameter("x", [BS, G], f32, isOutput=False)
    # columns [0, BS) = per-row scale permuted so tile q's rows are columns
    # [q*128, (q+1)*128); columns [BS, BS+G) = gene one-hot. One parameter ->
    # one DMA -> one wait semaphore for every matmul.
    tables = nc.declare_dram_parameter("tables", [C, BS + G], bf16, isOutput=False)
    out = nc.declare_dram_parameter("out", [BS, G], f32, isOutput=True)

    # row = p*QW + q: partition p holds 16 consecutive rows; tile q takes the
    # q-th row of every partition -> 38KB contiguous per partition per DMA.
    x_r = x[:, :].rearrange("(p q) g -> p q g", q=QW)
    out_r = out[:, :].rearrange("(p q) g -> p q g", q=QW)

    with TileContext(nc) as tc:
        with (
            tc.tile_pool(name="const", bufs=1) as const_pool,
            tc.tile_pool(name="xp", bufs=3) as xp,
            tc.tile_pool(name="pp", bufs=8, space="PSUM") as pp,
        ):
            tbl = const_pool.tile([C, BS + G], bf16, tag="tbl")
            nc.sync.dma_start(tbl[:], tables[:])

            for q in range(QW):  # 16 row groups
                for h in range(2):  # two half-rows
                    c0 = h * W
                    xt = xp.tile([P, W], f32, tag="xt")
                    nc.sync.dma_start(xt[:], x_r[:, q, c0 : c0 + W])
                    for off in range(0, W, MM):
                        w = min(MM, W - off)
                        m = pp.tile([P, MM], f32, tag="m")
                        nc.tensor.matmul(
                            m[:, :w],
                            tbl[:, q * P : (q + 1) * P],
                            tbl[:, BS + c0 + off : BS + c0 + off + w],
                            start=True,
                            stop=True,
                        )
                        nc.vector.tensor_tensor(
                            xt[:, off : off + w],
                            xt[:, off : off + w],
                            m[:, :w],
                            mybir.AluOpType.mult,
                        )
                    nc.sync.dma_start(out_r[:, q, c0 : c0 + W], xt[:])
    nc.finalize()  # Bacc.finalize -> compile() (wait splitting etc) + freeze
    return nc


def _host_tables(chrom_ids: np.ndarray) -> tuple[np.ndarray, np.ndarray]:
    """scale [B, 23] with values {0, 5.75}; onehot [23, G] - both float32."""
    import jax

    with jax.default_device(jax.devices("cpu")[0]):
        keys = jax.random.split(jax.random.key(42), B)
        sel = np.asarray(
            jax.vmap(lambda k: jax.random.permutation(k, C)[:ND])(keys)
        )  # [B, 4] int32
    drop = np.zeros((B, C), np.float32)
    drop[np.arange(B)[:, None], sel] = 1.0
    scale = (1.0 - drop) * np.float32(SCALE)  # [B, 23]
    onehot = (
        np.asarray(chrom_ids)[None, :] == np.arange(C, dtype=np.int32)[:, None]
    ).astype(np.float32)  # [23, G]
    return scale, onehot


def kernel(x: np.ndarray, chrom_ids: np.ndarray, **run_kwargs) -> np.ndarray:
    import ml_dtypes

    from concourse.bass_utils import run_bass_kernel_spmd

    x = np.asarray(x)
    scale, onehot = _host_tables(chrom_ids)
    bf16 = ml_dtypes.bfloat16
    onehot_b = onehot.astype(bf16)

    if "nc" not in _CACHED:
        _CACHED["nc"] = _build_nc()
    nc = _CACHED["nc"]

    in_maps = []
    for i in range(NCORES):
        sh = scale[i * BS : (i + 1) * BS]  # [2048, 23]
        # column q*128+p  <-  row p*QW+q of this shard (tile q = columns
        # [q*128, (q+1)*128) with partition index p)
        sp = sh.reshape(P, QW, C).transpose(2, 1, 0).reshape(C, BS)
        in_maps.append(
            {
                "x": np.ascontiguousarray(x[i * BS : (i + 1) * BS]),
                "tables": np.ascontiguousarray(
                    np.concatenate([sp.astype(bf16), onehot_b], axis=1)
                ),
            }
        )
    res = run_bass_kernel_spmd(nc, in_maps, core_ids=list(range(NCORES)), **run_kwargs)
    out = np.concatenate([np.asarray(r["out"]) for r in res.results], axis=0)
    if res.exec_time_ns is not None:
        kernel.last_exec_time_ns = res.exec_time_ns
    kernel.last_results = res
    return out


# revision 9
# speedup vs baseline: 1.2583x; 1.2583x over previous
"""ChromDropout kernel for one TRN2 chip (8 NeuronCores, data-parallel).

Math (training-mode ChromDropout):
    out[b, g] = x[b, g] * (1 - drop[b, chrom_ids[g]]) * (NUM_CHROMS / N_DROP)
where drop[b, :] marks 4 distinct chromosomes sampled per row with
jax.random.permutation(split(key(42), B)[b], 23)[:4].

Strategy:
  - Host (tiny): derive the per-row keep table exactly as the reference does
    (threefry is platform-deterministic) and a [23, G] one-hot of chrom_ids.
    Both are {0,1}-valued -> shipped as fp8 (exact), 21KB/partition in SBUF.
  - Device (all the heavy traffic): shard x row-wise across 8 cores
    (2048 rows each), processed as 16 tiles of 128 adjacent rows x full
    19064-gene width — each tile load/store is one fully contiguous 9.75MB
    DMA. Per tile, mask01[128, N] = keepT[23, 128].T @ onehot[23, N] on the
    TensorEngine (exact one-hot selection), then a single fused VectorEngine
    op out = (x * 5.75) * mask01 per chunk. Streaming, memory-bound.

The 5.75 rescale placement keeps bitwise equality with the reference:
(x*1.0)*5.75 and (x*5.75)*1.0 round identically; dropped genes give +-0.0
either way.
"""

import numpy as np

B = 16384
G = 19064
C = 23
ND = 4
SCALE = float(C) / float(ND)  # 5.75
NCORES = 8
BS = B // NCORES  # 2048 rows per core
P = 128
NT = BS // P  # 16 tiles of 128 adjacent rows
MM = 512  # matmul moving-free-dim chunk (one PSUM bank of f32)

_CACHED = {}


def _build_nc():
    import concourse.bacc as bacc
    import concourse.mybir as mybir
    from concourse.tile import TileContext

    f32 = mybir.dt.float32
    fp8 = mybir.dt.float8e4
    # Bacc (not raw Bass): its compile() runs move_matmul_waits_to_ldweights +
    # generate_event_semaphores, which split multi-semaphore waits down to the
    # 1-wait-per-instruction TRN2 ISA limit.
    nc = bacc.Bacc("TRN2", target_bir_lowering=False, debug=False)
    x = nc.declare_dram_parameter("x", [BS, G], f32, isOutput=False)
    # columns [0, BS) = per-row keep indicator (transposed, row-order columns),
    # [BS, BS+G) = gene one-hot. One parameter -> one DMA -> one wait
    # semaphore for every matmul.
    tables = nc.declare_dram_parameter("tables", [C, BS + G], fp8, isOutput=False)
    out = nc.declare_dram_parameter("out", [BS, G], f32, isOutput=True)

    x_t = x[:, :].rearrange("(t p) g -> t p g", p=P)
    out_t = out[:, :].rearrange("(t p) g -> t p g", p=P)

    with TileContext(nc) as tc:
        with (
            tc.tile_pool(name="const", bufs=1) as const_pool,
            tc.tile_pool(name="xp", bufs=2) as xp,
            tc.tile_pool(name="pp", bufs=8, space="PSUM") as pp,
        ):
            tbl = const_pool.tile([C, BS + G], fp8, tag="tbl")
            nc.sync.dma_start(tbl[:], tables[:])

            for t in range(NT):  # 16 tiles of 128 adjacent rows, full width
                xt = xp.tile([P, G], f32, tag="xt")
                nc.sync.dma_start(xt[:], x_t[t])
                for off in range(0, G, MM):
                    w = min(MM, G - off)
                    m = pp.tile([P, MM], f32, tag="m")
                    nc.tensor.matmul(
                        m[:, :w],
                        tbl[:, t * P : (t + 1) * P],
                        tbl[:, BS + off : BS + off + w],
                        start=True,
                        stop=True,
                    )
                    # out = (x * 5.75) * mask01, single DVE op
                    nc.vector.scalar_tensor_tensor(
                        xt[:, off : off + w],
                        xt[:, off : off + w],
                        SCALE,
                        m[:, :w],
                        mybir.AluOpType.mult,
                        mybir.AluOpType.mult,
                    )
                nc.sync.dma_start(out_t[t], xt[:])
    nc.finalize()  # Bacc.finalize -> compile() (wait splitting etc) + freeze
    return nc


def _host_tables(chrom_ids: np.ndarray) -> tuple[np.ndarray, np.ndarray]:
    """keep [B, 23] in {0,1}; onehot [23, G] in {0,1} - both float32."""
    import jax

    with jax.default_device(jax.devices("cpu")[0]):
        keys = jax.random.split(jax.random.key(42), B)
        sel = np.asarray(
            jax.vmap(lambda k: jax.random.permutation(k, C)[:ND])(keys)
        )  # [B, 4] int32
    keep = np.ones((B, C), np.float32)
    keep[np.arange(B)[:, None], sel] = 0.0
    onehot = (
        np.asarray(chrom_ids)[None, :] == np.arange(C, dtype=np.int32)[:, None]
    ).astype(np.float32)  # [23, G]
    return keep, onehot


def kernel(x: np.ndarray, chrom_ids: np.ndarray, **run_kwargs) -> np.ndarray:
    import ml_dtypes

    from concourse.bass_utils import run_bass_kernel_spmd

    x = np.asarray(x)
    keep, onehot = _host_tables(chrom_ids)
    fp8 = ml_dtypes.float8_e4m3
    onehot_8 = onehot.astype(fp8)

    if "nc" not in _CACHED:
        _CACHED["nc"] = _build_nc()
    nc = _CACHED["nc"]

    in_maps = []
    for i in range(NCORES):
        keep_t = keep[i * BS : (i + 1) * BS].T  # [23, 2048], row-order columns
        in_maps.append(
            {
                "x": np.ascontiguousarray(x[i * BS : (i + 1) * BS]),
                "tables": np.ascontiguousarray(
                    np.concatenate([keep_t.astype(fp8), onehot_8], axis=1)
                ),
            }
        )
    res = run_bass_kernel_spmd(nc, in_maps, core_ids=list(range(NCORES)), **run_kwargs)
    out = np.concatenate([np.asarray(r["out"]) for r in res.results], axis=0)
    if res.exec_time_ns is not None:
        kernel.last_exec_time_ns = res.exec_time_ns
    kernel.last_results = res
    return out
